# revision 1
# baseline (speedup 1.0000x reference)
"""Trainium2 Bass kernel for nn_BlockPiecewiseLinear (histogram_binning).

Math reformulation (validated vs the JAX reference to ~6e-7 rel):
    S    = softplus(slope)                      # [.., K+1]
    xs   = sort(x_pos, axis=-1)                 # [.., K]
    dS_j = S[j] - S[j-1]            (j = 1..K, stored at 0..K-1)
    step_j = 1[xs[j-1] <= q]
    A    = sum_j step_j * dS_j
    W    = sum_j step_j * dS_j * xs[j-1]
    slope_sel = (S[0]+EPS) + A
    out  = q*slope_sel - xs[0]*(S[0]+EPS) + xs[0] - W + y_bias

Sharding: pure data-parallel over the batch dim across 8 NeuronCores.
Per-core layout: rows (b,f) flattened; each SBUF tile is [128 part, G rows, K knots].
The 32-knot sort is an alternating-direction bitonic network (15 layers, 50
tensor_tensor min/max ops on AP-regular index subsets); everything else is
elementwise + free-dim reduces on DVE, softplus on ScalarE.
"""

import numpy as np

import concourse.bass as bass
import concourse.bacc as bacc
import concourse.mybir as mybir
import concourse.tile as tile
from concourse.bass_utils import run_bass_kernel_spmd

F32 = mybir.dt.float32
Alu = mybir.AluOpType
Act = mybir.ActivationFunctionType
AxX = mybir.AxisListType.X

B, F, K = 4096, 512, 32
KP1 = K + 1
EPS = 1e-3
NCORES = 8
P = 128
G = 128  # rows per partition per tile; P*G rows/tile


def _bitonic_layers(n=32):
    layers = []
    k = 2
    while k <= n:
        j = k // 2
        while j >= 1:
            layers.append((k, j))
            j //= 2
        k *= 2
    return layers  # 15 layers for n=32


def _emit_sort_layer(nc, cur, dst, kk, jj, g):
    """Alternating-direction bitonic layer (block size kk, distance jj).

    Index bits i = c*2k + d*k + m*2j + e*j + r; d selects sort direction.
    walrus lowers DVE operands as TENSOR3D (3 free dims after stride
    coalescing), so the ascending and descending halves must stay separate
    min/max ops: their fused output pattern would need 4 levels.
    """
    if kk < K and jj == kk // 2:
        # first sub-layer of each k-stage: the m dim is unit and g always
        # coalesces with c (32 = 2k * 16/k), so asc+desc fuse into one min
        # and one max op within TENSOR3D's 3-free-dim limit.
        # i = c*2k + d*k + e*j + r; min -> c*2k + d*(k+j) + r,
        # max -> j + c*2k + d*(k-j) + r
        cs = 16 // kk
        base_s = cur[:, :, :]
        base_d = dst[:, :, :]
        rlev = ([[1, jj]] if jj > 1 else [])
        in_ap = [base_s.ap[0], [2 * kk, g * cs], [kk, 2]] + rlev
        in_lo = bass.AP(tensor=base_s.tensor, offset=base_s.offset, ap=in_ap)
        in_hi = bass.AP(tensor=base_s.tensor, offset=base_s.offset + jj, ap=in_ap)
        out_min = bass.AP(tensor=base_d.tensor, offset=base_d.offset,
                          ap=[base_d.ap[0], [2 * kk, g * cs], [kk + jj, 2]] + rlev)
        out_max = bass.AP(tensor=base_d.tensor, offset=base_d.offset + jj,
                          ap=[base_d.ap[0], [2 * kk, g * cs], [kk - jj, 2]] + rlev)
        nc.vector.tensor_tensor(out=out_min, in0=in_lo, in1=in_hi, op=Alu.min)
        nc.vector.tensor_tensor(out=out_max, in0=in_lo, in1=in_hi, op=Alu.max)
    elif kk < K:
        cs = 16 // kk
        ms = kk // (2 * jj)
        vs = cur[:, :, :].rearrange(
            "p g (c d m e r) -> p g c d m e r", c=cs, d=2, m=ms, e=2, r=jj
        )
        vd = dst[:, :, :].rearrange(
            "p g (c d m e r) -> p g c d m e r", c=cs, d=2, m=ms, e=2, r=jj
        )
        a_lo = vs[:, :, :, 0, :, 0, :]
        a_hi = vs[:, :, :, 0, :, 1, :]
        nc.vector.tensor_tensor(out=vd[:, :, :, 0, :, 0, :], in0=a_lo, in1=a_hi, op=Alu.min)
        nc.vector.tensor_tensor(out=vd[:, :, :, 0, :, 1, :], in0=a_lo, in1=a_hi, op=Alu.max)
        d_lo = vs[:, :, :, 1, :, 0, :]
        d_hi = vs[:, :, :, 1, :, 1, :]
        nc.vector.tensor_tensor(out=vd[:, :, :, 1, :, 0, :], in0=d_lo, in1=d_hi, op=Alu.max)
        nc.vector.tensor_tensor(out=vd[:, :, :, 1, :, 1, :], in0=d_lo, in1=d_hi, op=Alu.min)
    else:
        ms = kk // (2 * jj)
        vs = cur[:, :, :].rearrange(
            "p g (m e r) -> p g m e r", m=ms, e=2, r=jj
        )
        vd = dst[:, :, :].rearrange(
            "p g (m e r) -> p g m e r", m=ms, e=2, r=jj
        )
        a_lo = vs[:, :, :, 0, :]
        a_hi = vs[:, :, :, 1, :]
        nc.vector.tensor_tensor(out=vd[:, :, :, 0, :], in0=a_lo, in1=a_hi, op=Alu.min)
        nc.vector.tensor_tensor(out=vd[:, :, :, 1, :], in0=a_lo, in1=a_hi, op=Alu.max)


def build_nc(nloc, g=G):
    rows_per_tile = P * g
    ntiles = nloc // rows_per_tile
    assert ntiles * rows_per_tile == nloc

    nc = bacc.Bacc("TRN2", target_bir_lowering=False, debug=False)
    x_d = nc.declare_dram_parameter("x", [nloc, K], F32, isOutput=False)
    sl_d = nc.declare_dram_parameter("sl", [nloc, KP1], F32, isOutput=False)
    q_d = nc.declare_dram_parameter("q", [nloc], F32, isOutput=False)
    yb_d = nc.declare_dram_parameter("yb", [P, g], F32, isOutput=False)
    out_d = nc.declare_dram_parameter("out", [nloc], F32, isOutput=True)
    ss_d = nc.declare_dram_parameter("ssel", [nloc], F32, isOutput=True)

    xv = x_d[:, :].rearrange("(t p g) k -> t p g k", p=P, g=g)
    slv = sl_d[:, :].rearrange("(t p g) k -> t p g k", p=P, g=g)
    qv = q_d[:].rearrange("(t p g) -> t p g", p=P, g=g)
    outv = out_d[:].rearrange("(t p g) -> t p g", p=P, g=g)
    ssv = ss_d[:].rearrange("(t p g) -> t p g", p=P, g=g)

    layers = _bitonic_layers(K)

    with tile.TileContext(nc) as tc:
        with (
            tc.tile_pool(name="pyb", bufs=1) as pyb,
            tc.tile_pool(name="px", bufs=2) as px,
            tc.tile_pool(name="psort", bufs=2) as psort,
            tc.tile_pool(name="psl", bufs=1) as psl,
            tc.tile_pool(name="pS", bufs=2) as pS,
            tc.tile_pool(name="ptmp", bufs=3) as ptmp,
            tc.tile_pool(name="psm", bufs=4) as psm,
            tc.tile_pool(name="pq", bufs=4) as pq,
            tc.tile_pool(name="pout", bufs=4) as pout,
        ):
            yb_t = pyb.tile([P, g], F32, tag="yb")
            nc.scalar.dma_start(out=yb_t[:, :], in_=yb_d[:, :])

            # stores of tile t-1 are emitted after tile t's softplus, so on
            # the in-order ACT queue exp/ln(t) aren't stuck behind a store
            # that waits on tile t-1's DVE epilogue
            fin = None
            for t in range(ntiles):
                x_t = px.tile([P, g, K], F32, tag="x")
                nc.scalar.dma_start(out=x_t[:, :, :], in_=xv[t])
                sl_t = psl.tile([P, g, KP1], F32, tag="sl")
                nc.scalar.dma_start(out=sl_t[:, :, :], in_=slv[t])
                q_t = pq.tile([P, g], F32, tag="q")
                nc.scalar.dma_start(out=q_t[:, :], in_=qv[t])

                # softplus = ln(1 + exp(x)); exp in-place on the slope tile
                nc.scalar.activation(
                    out=sl_t[:, :, :], in_=sl_t[:, :, :], func=Act.Exp
                )
                S_t = pS.tile([P, g, KP1], F32, tag="S")
                nc.scalar.activation(
                    out=S_t[:, :, :], in_=sl_t[:, :, :], func=Act.Ln, bias=1.0
                )

                # dS early so the ACT pipeline stays decoupled
                dS_t = ptmp.tile([P, g, K], F32, tag="w3")
                nc.vector.tensor_tensor(
                    out=dS_t[:, :, :], in0=S_t[:, :, 1:KP1], in1=S_t[:, :, 0:K],
                    op=Alu.subtract,
                )
                sm = psm.tile([P, g, 8], F32, tag="sm")
                s0p = sm[:, :, 2]
                nc.vector.tensor_scalar_add(s0p, S_t[:, :, 0], EPS)

                if fin is not None:
                    po_u, po_W, po_v, po_ot, po_st, po_o, po_s = fin
                    nc.vector.tensor_tensor(out=po_v, in0=po_u, in1=po_W, op=Alu.subtract)
                    nc.vector.tensor_tensor(out=po_ot[:, :], in0=po_v, in1=yb_t[:, :], op=Alu.add)
                    nc.scalar.dma_start(out=po_o, in_=po_ot[:, :])
                    nc.scalar.dma_start(out=po_s, in_=po_st[:, :])
                    fin = None

                # ---- bitonic sort of the K knots (ascending) ----
                cur = x_t
                for kk, jj in layers:
                    dst = psort.tile([P, g, K], F32, tag="sort")
                    _emit_sort_layer(nc, cur, dst, kk, jj, g)
                    cur = dst
                xs_t = cur  # sorted ascending [P, g, K]

                # ---- knot-dim elementwise + reduces (DVE) ----
                step_t = ptmp.tile([P, g, K], F32, tag="w3")
                xs_full = xs_t[:, :, :]
                q2ap = q_t[:, :]
                qb = bass.AP(
                    tensor=q2ap.tensor,
                    offset=q2ap.offset,
                    ap=[q2ap.ap[0], q2ap.ap[1], [0, K]],
                )
                nc.vector.tensor_tensor(
                    out=step_t[:, :, :], in0=xs_full, in1=qb, op=Alu.is_le
                )
                m_t = ptmp.tile([P, g, K], F32, tag="w3")
                nc.vector.tensor_tensor(
                    out=m_t[:, :, :], in0=dS_t[:, :, :], in1=step_t[:, :, :],
                    op=Alu.mult,
                )
                w_t = ptmp.tile([P, g, K], F32, tag="w3")
                nc.vector.tensor_tensor(
                    out=w_t[:, :, :], in0=m_t[:, :, :], in1=xs_full, op=Alu.mult
                )

                # epilogue ops that don't need W go between the w-mult and
                # the W-reduce so the DVE pipe drain of w is hidden
                A = sm[:, :, 0]
                W = sm[:, :, 1]
                nc.vector.tensor_reduce(out=A, in_=m_t[:, :, :], axis=AxX, op=Alu.add)

                # ---- epilogue on [P, g] (DVE):
                #   ssel = s0p + A
                #   out  = q*ssel - xmin*s0p + xmin - W + yb
                q2 = q_t[:, :]
                xmin = xs_t[:, :, 0]
                u = sm[:, :, 4]
                v = sm[:, :, 5]
                r = sm[:, :, 6]
                out_t = pout.tile([P, g], F32, tag="out")
                ss_t = pout.tile([P, g], F32, tag="ss")
                nc.vector.tensor_tensor(out=ss_t[:, :], in0=s0p, in1=A, op=Alu.add)
                nc.vector.tensor_tensor(out=u, in0=q2, in1=ss_t[:, :], op=Alu.mult)
                nc.vector.tensor_tensor(out=v, in0=xmin, in1=s0p, op=Alu.mult)
                nc.vector.tensor_tensor(out=r, in0=u, in1=v, op=Alu.subtract)
                nc.vector.tensor_tensor(out=u, in0=r, in1=xmin, op=Alu.add)
                nc.vector.tensor_reduce(out=W, in_=w_t[:, :, :], axis=AxX, op=Alu.add)
                # the two W-dependent ops are deferred into the next
                # iteration (after its dS/s0p ops) to hide W's pipe drain
                fin = (u, W, v, out_t, ss_t, outv[t], ssv[t])

            po_u, po_W, po_v, po_ot, po_st, po_o, po_s = fin
            nc.vector.tensor_tensor(out=po_v, in0=po_u, in1=po_W, op=Alu.subtract)
            nc.vector.tensor_tensor(out=po_ot[:, :], in0=po_v, in1=yb_t[:, :], op=Alu.add)
            nc.scalar.dma_start(out=po_o, in_=po_ot[:, :])
            nc.scalar.dma_start(out=po_s, in_=po_st[:, :])
    nc.compile()
    return nc


_NC_CACHE = {}


def _get_nc(nloc, g=G):
    key = (nloc, g)
    if key not in _NC_CACHE:
        _NC_CACHE[key] = build_nc(nloc, g)
    return _NC_CACHE[key]


def kernel(inputs, x_pos, slope, y_bias):
    inputs = np.ascontiguousarray(np.asarray(inputs, dtype=np.float32))
    x_pos = np.ascontiguousarray(np.asarray(x_pos, dtype=np.float32))
    slope = np.ascontiguousarray(np.asarray(slope, dtype=np.float32))
    y_bias = np.ascontiguousarray(np.asarray(y_bias, dtype=np.float32))

    b, f = inputs.shape
    bloc = b // NCORES
    nloc = bloc * f
    nc = _get_nc(nloc)

    # y_bias expanded to the [P, G] per-tile layout: row (p, g) has f = (p*G+g) % F
    yb_exp = np.ascontiguousarray(np.tile(y_bias[:, 0], (P * G) // f).reshape(P, G))

    in_maps = []
    for c in range(NCORES):
        sl_b = slice(c * bloc, (c + 1) * bloc)
        in_maps.append(
            {
                "x": x_pos[sl_b].reshape(nloc, K),
                "sl": slope[sl_b].reshape(nloc, KP1),
                "q": inputs[sl_b].reshape(nloc),
                "yb": yb_exp,
            }
        )

    res = run_bass_kernel_spmd(nc, in_maps, list(range(NCORES)))
    outs = np.concatenate(
        [res.results[c]["out"].reshape(bloc, f) for c in range(NCORES)], axis=0
    )
    ssel = np.concatenate(
        [res.results[c]["ssel"].reshape(bloc, f) for c in range(NCORES)], axis=0
    )
    return outs, ssel



# revision 2
# speedup vs baseline: 1.0046x; 1.0046x over previous
"""Trainium2 Bass kernel v4 (v2 + fused epilogue) for nn_BlockPiecewiseLinear (histogram_binning).

Math (same reformulation as baseline, validated to ~4e-6):
    S    = softplus(slope)                      # [.., K+1]
    xs   = sort(x_pos, axis=-1)                 # [.., K]  (fp16)
    dS_r = S[r+1] - S[r]            (r = 0..K-1)
    c    = #{k: x_k <= q}           (EXACT fp32 compare on unsorted x)
    step'_r = 1[r < c]              (prefix mask from exact count)
    A    = sum_r step'_r * dS_r
    W    = sum_r step'_r * dS_r * xs_r
    ssel = (S[0]+EPS) + A
    out  = q*ssel - xs[0]*(S[0]+EPS) + xs[0] - W + y_bias

v2 layout: knot-major fp16 tiles [P, K, G] so every bitonic layer's
tensor_tensor runs in the DVE 2x packed mode (fp32 row-major is stuck at
1x).  Sort is the all-ascending bitonic variant (reversal merge + halving
layers) -> every layer is exactly 2 ops.  The exact count c is computed in
fp32 on the row-major x (contiguous), so fp16 rounding can never flip a
segment decision (which would cause O(dS) errors).  ScalarE does softplus
(exp + transposed ln) and the x transpose-cast; DVE does the rest.
"""

import numpy as np

import concourse.bass as bass
import concourse.bacc as bacc
import concourse.mybir as mybir
import concourse.tile as tile
from concourse.bass_utils import run_bass_kernel_spmd

F32 = mybir.dt.float32
F16 = mybir.dt.float16
Alu = mybir.AluOpType
Act = mybir.ActivationFunctionType

B, F, K = 4096, 512, 32
KP1 = K + 1
EPS = 1e-3
NCORES = 8
P = 128
G = 128  # rows per partition per tile


def _ap(t, off_elems, dims):
    """AP on tile-view t with extra element offset and free dims list."""
    v = t[:, :, :] if len(t.shape) == 3 else t[:, :]
    return bass.AP(tensor=v.tensor, offset=v.offset + off_elems, ap=[v.ap[0]] + dims)


def _sort_layers():
    """All-ascending bitonic: for k in 1,2,4,8,16: reversal layer (i <-> 2k-1-i
    in 2k-blocks) then halving layers j=k/2..1 (i <-> i+j in 2j-blocks)."""
    layers = []
    k = 1
    while k < K:
        layers.append(("rev", k))
        j = k // 2
        while j >= 1:
            layers.append(("half", j))
            j //= 2
        k *= 2
    return layers  # 15 layers


def build_nc(nloc, g=G):
    rows_per_tile = P * g
    ntiles = nloc // rows_per_tile
    assert ntiles * rows_per_tile == nloc

    nc = bacc.Bacc("TRN2", target_bir_lowering=False, debug=False)
    x_d = nc.declare_dram_parameter("x", [nloc, K], F32, isOutput=False)
    sl_d = nc.declare_dram_parameter("sl", [nloc, KP1], F32, isOutput=False)
    q_d = nc.declare_dram_parameter("q", [nloc], F32, isOutput=False)
    yb_d = nc.declare_dram_parameter("yb", [P, g], F32, isOutput=False)
    io_d = nc.declare_dram_parameter("io", [P, K * g], F16, isOutput=False)
    out_d = nc.declare_dram_parameter("out", [nloc], F32, isOutput=True)
    ss_d = nc.declare_dram_parameter("ssel", [nloc], F32, isOutput=True)

    xv = x_d[:, :].rearrange("(t p g) k -> t p g k", p=P, g=g)
    slv = sl_d[:, :].rearrange("(t p g) k -> t p g k", p=P, g=g)
    qv = q_d[:].rearrange("(t p g) -> t p g", p=P, g=g)
    outv = out_d[:].rearrange("(t p g) -> t p g", p=P, g=g)
    ssv = ss_d[:].rearrange("(t p g) -> t p g", p=P, g=g)

    layers = _sort_layers()

    with tile.TileContext(nc) as tc:
        with (
            tc.tile_pool(name="pcst", bufs=1) as pcst,
            tc.tile_pool(name="px", bufs=2) as px,
            tc.tile_pool(name="psl", bufs=2) as psl,
            tc.tile_pool(name="pq", bufs=2) as pq,
            tc.tile_pool(name="pS", bufs=2) as pS,
            tc.tile_pool(name="pxm", bufs=2) as pxm,
            tc.tile_pool(name="psort", bufs=3) as psort,
            tc.tile_pool(name="pst0", bufs=1) as pst0,
            tc.tile_pool(name="ptc", bufs=1) as ptc,
            tc.tile_pool(name="pstp", bufs=1) as pstp,
            tc.tile_pool(name="pdS", bufs=1) as pdS,
            tc.tile_pool(name="pmw", bufs=1) as pmw,
            tc.tile_pool(name="ptm", bufs=1) as ptm,
            tc.tile_pool(name="psm", bufs=1) as psm,
            tc.tile_pool(name="pout", bufs=3) as pout,
        ):
            yb_t = pcst.tile([P, g], F32, tag="yb")
            nc.sync.dma_start(out=yb_t[:, :], in_=yb_d[:, :])
            io_t = pcst.tile([P, K, g], F16, tag="io")
            nc.sync.dma_start(
                out=io_t[:, :, :], in_=io_d[:, :].rearrange("p (k g) -> p k g", g=g)
            )

            for t in range(ntiles):
                # ---------------- DMA loads ----------------
                x_t = px.tile([P, g, K], F32, tag="x")
                nc.sync.dma_start(out=x_t[:, :, :], in_=xv[t])
                sl_t = psl.tile([P, g, KP1], F32, tag="sl")
                nc.scalar.dma_start(out=sl_t[:, :, :], in_=slv[t])
                q_t = pq.tile([P, g], F32, tag="q")
                nc.sync.dma_start(out=q_t[:, :], in_=qv[t])

                # ---------------- ScalarE: softplus + x transpose-cast ----
                # S = ln(1 + exp(sl)); ln reads row-major, writes knot-major f16
                nc.scalar.activation(out=sl_t[:, :, :], in_=sl_t[:, :, :], func=Act.Exp)
                S_t = pS.tile([P, KP1, g], F16, tag="S")
                nc.scalar.activation(
                    out=_ap(S_t, 0, [[g, KP1], [1, g]]),
                    in_=_ap(sl_t, 0, [[1, KP1], [KP1, g]]),
                    func=Act.Ln,
                    bias=1.0,
                )
                # x f32 row-major -> f16 knot-major
                xm_t = pxm.tile([P, K, g], F16, tag="xm")
                nc.scalar.activation(
                    out=_ap(xm_t, 0, [[g, K], [1, g]]),
                    in_=_ap(x_t, 0, [[1, K], [K, g]]),
                    func=Act.Copy,
                )

                # ---------------- DVE: exact count c ----------------
                # step0 (row-major, f16) = 1[x <= q]; fp32 compare
                st0 = pst0.tile([P, g, K], F16, tag="st0")
                nc.vector.tensor_tensor(
                    out=st0[:, :, :],
                    in0=x_t[:, :, :],
                    in1=_ap(q_t, 0, [[1, g], [0, K]]),
                    op=Alu.is_le,
                )
                # 5-level pair tree over K (innermost) -> c [P, g] f16
                # levels write to disjoint offsets of tc1: L1@0(w16) L2@16(w8)
                # L3@24(w4) L4@28(w2); L5 -> contiguous c_sm [P, g]
                tc1 = ptc.tile([P, g, K], F16, tag="tc")
                nc.vector.tensor_tensor(
                    out=_ap(tc1, 0, [[K, g], [1, 16]]),
                    in0=_ap(st0, 0, [[K, g], [1, 16]]),
                    in1=_ap(st0, 16, [[K, g], [1, 16]]),
                    op=Alu.add,
                )
                nc.vector.tensor_tensor(
                    out=_ap(tc1, 16, [[K, g], [1, 8]]),
                    in0=_ap(tc1, 0, [[K, g], [1, 8]]),
                    in1=_ap(tc1, 8, [[K, g], [1, 8]]),
                    op=Alu.add,
                )
                nc.vector.tensor_tensor(
                    out=_ap(tc1, 24, [[K, g], [1, 4]]),
                    in0=_ap(tc1, 16, [[K, g], [1, 4]]),
                    in1=_ap(tc1, 20, [[K, g], [1, 4]]),
                    op=Alu.add,
                )
                nc.vector.tensor_tensor(
                    out=_ap(tc1, 28, [[K, g], [1, 2]]),
                    in0=_ap(tc1, 24, [[K, g], [1, 2]]),
                    in1=_ap(tc1, 26, [[K, g], [1, 2]]),
                    op=Alu.add,
                )
                c_sm = ptc.tile([P, g], F16, tag="csm")
                nc.vector.tensor_tensor(
                    out=c_sm[:, :],
                    in0=_ap(tc1, 28, [[K, g]]),
                    in1=_ap(tc1, 29, [[K, g]]),
                    op=Alu.add,
                )

                # step' (knot-major) = 1[iota < c]
                stp = pstp.tile([P, K, g], F16, tag="stp")
                nc.vector.tensor_tensor(
                    out=stp[:, :, :],
                    in0=io_t[:, :, :],
                    in1=bass.AP(
                        tensor=c_sm.tensor,
                        offset=c_sm[:, :].offset,
                        ap=[c_sm[:, :].ap[0], [0, K], [1, g]],
                    ),
                    op=Alu.is_lt,
                )

                # ---------------- DVE: bitonic sort (knot-major f16) ------
                cur = xm_t
                for kind, kk in layers:
                    dst = psort.tile([P, K, g], F16, tag="srt")
                    if kind == "rev":
                        bsz = 2 * kk
                        nb = K // bsz
                        in_lo = _ap(cur, 0, [[bsz * g, nb], [1, kk * g]])
                        if kk == 1:
                            in_hi = _ap(cur, g, [[bsz * g, nb], [1, g]])
                            o_max = _ap(dst, g, [[bsz * g, nb], [1, g]])
                        else:
                            in_hi = _ap(cur, (bsz - 1) * g, [[bsz * g, nb], [-g, kk], [1, g]])
                            o_max = _ap(dst, (bsz - 1) * g, [[bsz * g, nb], [-g, kk], [1, g]])
                        o_min = _ap(dst, 0, [[bsz * g, nb], [1, kk * g]])
                    else:
                        jj = kk
                        bsz = 2 * jj
                        nb = K // bsz
                        in_lo = _ap(cur, 0, [[bsz * g, nb], [1, jj * g]])
                        in_hi = _ap(cur, jj * g, [[bsz * g, nb], [1, jj * g]])
                        o_min = _ap(dst, 0, [[bsz * g, nb], [1, jj * g]])
                        o_max = _ap(dst, jj * g, [[bsz * g, nb], [1, jj * g]])
                    nc.vector.tensor_tensor(out=o_min, in0=in_lo, in1=in_hi, op=Alu.min)
                    nc.vector.tensor_tensor(out=o_max, in0=in_lo, in1=in_hi, op=Alu.max)
                    cur = dst
                xs_t = cur  # sorted ascending, [P, K, g] f16

                # ---------------- DVE: dS, m, w, tree-reduce ----------------
                dS_t = pdS.tile([P, K, g], F16, tag="dS")
                nc.vector.tensor_tensor(
                    out=dS_t[:, :, :],
                    in0=_ap(S_t, g, [[1, K * g]]),
                    in1=_ap(S_t, 0, [[1, K * g]]),
                    op=Alu.subtract,
                )
                mw = pmw.tile([P, 2, K, g], F16, tag="mw")
                m_v = _ap(mw, 0, [[1, K * g]])
                w_v = _ap(mw, K * g, [[1, K * g]])
                nc.vector.tensor_tensor(out=m_v, in0=stp[:, :, :], in1=dS_t[:, :, :], op=Alu.mult)
                nc.vector.tensor_tensor(out=w_v, in0=m_v, in1=xs_t[:, :, :], op=Alu.mult)
                # 5-level tree over K for both halves; last 2 levels in f32
                t16 = ptm.tile([P, 2, 16, g], F16, tag="t16")
                nc.vector.tensor_tensor(
                    out=_ap(t16, 0, [[16 * g, 2], [1, 16 * g]]),
                    in0=_ap(mw, 0, [[K * g, 2], [1, 16 * g]]),
                    in1=_ap(mw, 16 * g, [[K * g, 2], [1, 16 * g]]),
                    op=Alu.add,
                )
                t8 = ptm.tile([P, 2, 8, g], F16, tag="t8")
                nc.vector.tensor_tensor(
                    out=_ap(t8, 0, [[8 * g, 2], [1, 8 * g]]),
                    in0=_ap(t16, 0, [[16 * g, 2], [1, 8 * g]]),
                    in1=_ap(t16, 8 * g, [[16 * g, 2], [1, 8 * g]]),
                    op=Alu.add,
                )
                t4 = ptm.tile([P, 2, 4, g], F16, tag="t4")
                nc.vector.tensor_tensor(
                    out=_ap(t4, 0, [[4 * g, 2], [1, 4 * g]]),
                    in0=_ap(t8, 0, [[8 * g, 2], [1, 4 * g]]),
                    in1=_ap(t8, 4 * g, [[8 * g, 2], [1, 4 * g]]),
                    op=Alu.add,
                )
                t2 = psm.tile([P, 2, 2, g], F32, tag="t2")
                nc.vector.tensor_tensor(
                    out=_ap(t2, 0, [[2 * g, 2], [1, 2 * g]]),
                    in0=_ap(t4, 0, [[4 * g, 2], [1, 2 * g]]),
                    in1=_ap(t4, 2 * g, [[4 * g, 2], [1, 2 * g]]),
                    op=Alu.add,
                )
                t1 = psm.tile([P, 2, 1, g], F32, tag="t1")
                nc.vector.tensor_tensor(
                    out=_ap(t1, 0, [[g, 2], [1, g]]),
                    in0=_ap(t2, 0, [[2 * g, 2], [1, g]]),
                    in1=_ap(t2, g, [[2 * g, 2], [1, g]]),
                    op=Alu.add,
                )
                A_v = _ap(t1, 0, [[1, g]])
                W_v = _ap(t1, g, [[1, g]])

                # ---------------- epilogue (fused via STT) -----------------
                # ssel = (S0 + EPS) + A ; t1e = (S0 + (EPS-1)) * xmin
                # out  = q*ssel - t1e - W + yb
                sm = psm.tile([P, 2, g], F32, tag="sm")
                ss_t = pout.tile([P, g], F32, tag="ss")
                nc.vector.scalar_tensor_tensor(
                    out=ss_t[:, :], in0=_ap(S_t, 0, [[1, g]]), scalar=EPS,
                    in1=A_v, op0=Alu.add, op1=Alu.add,
                )
                t1e = _ap(sm, 0, [[1, g]])
                nc.vector.scalar_tensor_tensor(
                    out=t1e, in0=_ap(S_t, 0, [[1, g]]), scalar=EPS - 1.0,
                    in1=_ap(xs_t, 0, [[1, g]]), op0=Alu.add, op1=Alu.mult,
                )
                u = _ap(sm, g, [[1, g]])
                nc.vector.tensor_tensor(out=u, in0=q_t[:, :], in1=ss_t[:, :], op=Alu.mult)
                nc.vector.tensor_tensor(out=u, in0=u, in1=t1e, op=Alu.subtract)
                nc.vector.tensor_tensor(out=u, in0=u, in1=W_v, op=Alu.subtract)
                out_t = pout.tile([P, g], F32, tag="out")
                nc.vector.tensor_tensor(
                    out=out_t[:, :], in0=u, in1=yb_t[:, :], op=Alu.add
                )

                # ---------------- stores ----------------
                nc.sync.dma_start(out=outv[t], in_=out_t[:, :])
                nc.scalar.dma_start(out=ssv[t], in_=ss_t[:, :])
    nc.compile()
    return nc


_NC_CACHE = {}


def _get_nc(nloc, g=G):
    key = (nloc, g)
    if key not in _NC_CACHE:
        _NC_CACHE[key] = build_nc(nloc, g)
    return _NC_CACHE[key]


def make_iota():
    io = np.broadcast_to(
        np.arange(K, dtype=np.float16)[None, :, None], (P, K, G)
    )
    return np.ascontiguousarray(io.reshape(P, K * G))


def make_in_maps(inputs, x_pos, slope, y_bias):
    b, f = inputs.shape
    bloc = b // NCORES
    nloc = bloc * f
    yb_exp = np.ascontiguousarray(
        np.tile(y_bias.astype(np.float32)[:, 0], (P * G) // f).reshape(P, G)
    )
    io = make_iota()
    in_maps = []
    for c in range(NCORES):
        sl_b = slice(c * bloc, (c + 1) * bloc)
        in_maps.append(
            {
                "x": np.ascontiguousarray(x_pos[sl_b].astype(np.float32).reshape(nloc, K)),
                "sl": np.ascontiguousarray(slope[sl_b].astype(np.float32).reshape(nloc, KP1)),
                "q": np.ascontiguousarray(inputs[sl_b].astype(np.float32).reshape(nloc)),
                "yb": yb_exp,
                "io": io,
            }
        )
    return in_maps, bloc, nloc


def kernel(inputs, x_pos, slope, y_bias):
    inputs = np.ascontiguousarray(np.asarray(inputs, dtype=np.float32))
    x_pos = np.ascontiguousarray(np.asarray(x_pos, dtype=np.float32))
    slope = np.ascontiguousarray(np.asarray(slope, dtype=np.float32))
    y_bias = np.ascontiguousarray(np.asarray(y_bias, dtype=np.float32))

    in_maps, bloc, nloc = make_in_maps(inputs, x_pos, slope, y_bias)
    b, f = inputs.shape
    nc = _get_nc(nloc)
    res = run_bass_kernel_spmd(nc, in_maps, list(range(NCORES)))
    outs = np.concatenate(
        [res.results[c]["out"].reshape(bloc, f) for c in range(NCORES)], axis=0
    )
    ssel = np.concatenate(
        [res.results[c]["ssel"].reshape(bloc, f) for c in range(NCORES)], axis=0
    )
    return outs, ssel
